# revision 64
# baseline (speedup 1.0000x reference)
"""Trainium2 Bass kernel for GRNNTransformSimple (bottom-up binary-tree GRNN).

Computation (per jet): heap-layout complete binary tree, DEPTH=14.
  u_k   = relu(contents_k @ Wu + bu)                         (all nodes)
  emb_k = u_k                                                (leaves)
  emb_k = relu(hL @ Wh[:64] + hR @ Wh[64:128] + u_k @ Wh[128:] + bh)  (inner)
Output: root emb, [B, 64].

Mapping (8 NeuronCores, data-parallel over B=128 jets, 16 jets/core):
 - 2 jets packed per 128 SBUF partitions (jet A on partitions 0-63, jet B on
   64-127) with block-diagonal weights -> all engines run 128 partitions wide.
 - fc_u biases folded into the matmul via a constant-one input row (K=18).
 - The "irregular" child gather is regular for arange children: children of
   level-i node j are nodes 2j, 2j+1 of level i+1, i.e. a stride-2 column
   slice of the level-(i+1) embedding buffer.

Performance notes (measured on TRN2; matmul issue cost is ~0.42 ns per
output column for bf16 K=128 AND for fp8 DoubleRow — which contracts two
K=128 halves in one pass; act relus are limited to ScalarE+VectorE at
~2.0 col/ns combined since GPSIMD and DMA cannot touch PSUM, and PSUM
reads lock the DVE to its 1x mode):
 - fc_h levels 12..4 fuse the hL/hR matmuls into ONE fp8e4m3 DoubleRow
   matmul: the moving AP's two "halves" are the even/odd emb columns of
   the level below (a stride-2 rearrange of the existing layout), the
   stationary is [blockdiag(WhL) | blockdiag(WhR)]. 2 PE cycles/col
   instead of 3; the u path and levels 3..0 stay bf16. Per tile the U
   matmul issues FIRST (start=True) and the DR accumulates (stop=True):
   U's moving data is a round old while DR reads the just-written level
   below, so the U work hides in the shadow of that act completing. emb at levels
   13..5 is stored fp8 (only ever read as DR moving data); quantization
   noise injected below level ~4 washes out through the tree (measured
   rel_rms 4.8e-3 vs 3.8e-3 all-bf16, tolerance 2e-2).
 - fc_u is output-bound (each 128-partition psum column = 2 jets x 64H,
   full density) -> stays bf16; DoubleRow at K_phys=18 measured 2x SLOWER
   per output column, so it is NOT used for fc_u. Stationaries stay
   zero-padded to full K=128 ("dense-label") to keep the DVFS governor
   from downclocking on low-occupancy matmuls.
 - Head: ~7.3us fixed preamble before any DMA trigger can fire plus
   ~3.2us fixed first-transfer latency per queue -> first data lands
   ~11us in regardless of size. Each of the three DMA-capable queues
   (sync/scalar/gpsimd) delivers a stream of 512-col chunks in exactly
   the PE's consumption order (pair-0 h0-halves on sync, h1-halves on
   gpsimd, weights on scalar), so the PE never re-stalls after the first
   landing. Steady-state c4[p] prefetch is split sync/gpsimd (a single
   queue moves ~100GB/s = most of a round for the 1MB pair tile).
 - Blended emission: h-tile of pair p-1 first in each slot, then 2
   u-tiles of pair p; adjacent same-strip u-pairs share one LDWEIGHTS
   via dedup (a weight switch costs a ~100ns PE issue bubble; fusing
   MORE tiles per stationary measured slower - psum pool pressure).
 - One shared 4-deep PSUM pool of 1024-col tiles (2048-col tiles with
   bufs=2 measured catastrophically slower: the 2-deep pipeline
   lockstepped the PE to act latency).
 - Greedy ns-balanced act assignment (scalar 120+0.97/col, vector
   140+1.06/col); tiny tail levels (<=6) pin each column-chain's acts
   to one engine (chain%2) so lineages run in parallel without
   cross-engine semaphore hops inside a lineage.
 - Tail: pair-7 h units, utop u groups (level-9 inputs first) and the
   level 9..8 chains hand-interleaved so the big tail levels run in the
   act-latency shadow of the utop round; levels 7..1 as two independent
   column-chains (chain c of level i feeds chain c of level i-1), root
   act + out DMA on scalar.
"""

import os
import sys

sys.path.insert(0, "/opt/trn_rl_repo")

import ml_dtypes
import numpy as np

DEPTH = 14
B = 128
F = 8
H = 64
N_NODES = 2**DEPTH - 1  # 16383
N_INNER = 2 ** (DEPTH - 1) - 1  # 8191
N_CORES = 8
JPC = 16  # jets per core
NPAIR = 8  # jet pairs per core

BF16 = ml_dtypes.bfloat16
E4M3 = ml_dtypes.float8_e4m3

# u_stream layout per pair (columns): levels 10,11,12 inner nodes in heap
# order (bf16 "usm" tile, 7168 cols), then all leaves in heap order (fp8
# "usl" tile, 8192 cols; leaves only feed the level-12 DoubleRow LR matmul).
UB10, UB11, UB12 = 0, 1024, 3072  # level bases inside usm
USM = 7168
USL = 8192
USTREAM = 15360  # 1024 + 2048 + 4096 + 8192
NGRP = 15  # 15 groups x 1024 cols
# u_top: levels 0..9, column order [level][pair][node]
UTOP_COLS = 8184  # 8 * 1023
UTOP_PAD = 8192


def _np_reference(contents, children, Wu, bu, Wh, bh):
    emb = None
    for i in range(DEPTH - 1, -1, -1):
        off, n = 2**i - 1, 2**i
        u = np.maximum(contents[:, off : off + n] @ Wu + bu, 0)
        if emb is None:
            emb = u
        else:
            ch = children[off : off + n] - 2 * off
            hL = emb[:, ch[:, 0]]
            hR = emb[:, ch[:, 1]]
            emb = np.maximum(
                hL @ Wh[:H] + hR @ Wh[H : 2 * H] + u @ Wh[2 * H :] + bh, 0
            )
    return emb.reshape(emb.shape[0], -1).astype(np.float32)


def _prep_core_inputs(contents):
    """contents: [16, 16383, 8] f32 for one core.
    Returns dict of per-core device input arrays."""
    c4 = np.zeros((NPAIR, 128, 4096), dtype=BF16)
    big_T = np.ascontiguousarray(
        np.transpose(contents[:, 1023:16383, :], (0, 2, 1))
    )  # [16, 8, 15360]
    for p in range(NPAIR):
        S = np.empty((18, USTREAM), dtype=np.float32)
        S[0:8] = big_T[2 * p]
        S[8] = 1.0
        S[9:17] = big_T[2 * p + 1]
        S[17] = 1.0
        Sb = S.astype(BF16)
        for g in range(NGRP):
            t = g % 4
            cc = 1024 * (g // 4)
            c4[p, 32 * t : 32 * t + 18, cc : cc + 1024] = Sb[
                :, 1024 * g : 1024 * (g + 1)
            ]

    # u_top stream: levels 0..9, [level][pair][node]
    tops = np.empty((18, UTOP_COLS), dtype=np.float32)
    colptr = 0
    cT = np.transpose(contents, (0, 2, 1))  # [16, 8, 16383]
    for i in range(10):
        off, n = 2**i - 1, 2**i
        for p in range(NPAIR):
            tops[0:8, colptr : colptr + n] = cT[2 * p][:, off : off + n]
            tops[8, colptr : colptr + n] = 1.0
            tops[9:17, colptr : colptr + n] = cT[2 * p + 1][:, off : off + n]
            tops[17, colptr : colptr + n] = 1.0
            colptr += n
    assert colptr == UTOP_COLS
    ctop = np.zeros((128, 2048), dtype=BF16)
    tb = np.zeros((18, UTOP_PAD), dtype=BF16)
    tb[:, :UTOP_COLS] = tops.astype(BF16)
    for g in range(8):
        t = g % 4
        cc = 1024 * (g // 4)
        ctop[32 * t : 32 * t + 18, cc : cc + 1024] = tb[:, 1024 * g : 1024 * (g + 1)]
    return {"c4": c4, "ctop": ctop}


def _prep_weights(Wu, bu, Wh, bh):
    wu2 = np.zeros((18, 128), dtype=np.float32)
    wu2[0:8, 0:64] = Wu
    wu2[8, 0:64] = bu
    wu2[9:17, 64:128] = Wu
    wu2[17, 64:128] = bu
    # Four full-K stationaries (one per 32-row strip): rows outside the
    # strip are zero so the other strips' data in the moving columns
    # contributes nothing. Full-K keeps the mm "dense" (128x128) from the
    # DVFS governor's perspective.
    wu_dram = np.zeros((4, 128, 128), dtype=BF16)
    for t in range(4):
        wu_dram[t, 32 * t : 32 * t + 18, :] = wu2.astype(BF16)
    wu_dram = wu_dram.transpose(1, 0, 2).reshape(128, 512)

    def blockdiag(Wx, dt=BF16):
        out = np.zeros((128, 128), dtype=np.float32)
        out[0:64, 0:64] = Wx
        out[64:128, 64:128] = Wx
        return out.astype(dt)

    whl = blockdiag(Wh[0:H])
    whr = blockdiag(Wh[H : 2 * H])
    whu = blockdiag(Wh[2 * H : 3 * H])
    # fp8 DoubleRow stationary for levels 12..4: half0 = blockdiag(WhL),
    # half1 = blockdiag(WhR); the two halves contract the even/odd emb
    # columns of the level below in a single PE pass.
    whlr = np.concatenate(
        [blockdiag(Wh[0:H], E4M3), blockdiag(Wh[H : 2 * H], E4M3)], axis=1
    )
    bh2 = np.concatenate([bh, bh]).astype(np.float32).reshape(128, 1)
    return {
        "wu": wu_dram,
        "whl": whl,
        "whr": whr,
        "whu": whu,
        "whlr": whlr,
        "bh2": bh2,
    }


def _dedup_ldweights(nc):
    """Delete an LDWEIGHTS whose signature matches the previous PE weight
    load when only instructions that cannot disturb the stationary operand
    (MATMULs, NoOps, semaphore ops) execute in between: the PE keeps the
    stationary resident, so load-once-matmul-many is safe. Sync info of
    deleted loads is merged into the following PE instruction."""
    n_del = 0
    transparent = ("InstMatmult", "InstNoOp", "InstEventSemaphore")
    for f in nc.m.functions:
        for bb in f.blocks:
            last_sig = None
            pending_sync = None
            out = []
            for inst in bb.instructions:
                tn = type(inst).__name__
                if str(getattr(inst, "engine", "")) == "EngineType.PE":
                    if tn == "InstLdweights":
                        a = inst.ins[0]
                        sig = (
                            getattr(a, "memref", None),
                            getattr(a, "offset", None),
                            str(getattr(a, "ap", None)),
                            str(inst.tile_position),
                            str(inst.tile_size),
                            str(inst.perf_mode),
                            str(inst.is_transpose),
                        )
                        if sig == last_sig:
                            n_del += 1
                            si = inst.sync_info
                            if si is not None and (si.on_wait or si.on_update):
                                if pending_sync is None:
                                    pending_sync = ([], [])
                                pending_sync[0].extend(si.on_wait)
                                pending_sync[1].extend(si.on_update)
                            continue  # drop this instruction
                        last_sig = sig
                    elif tn not in transparent:
                        last_sig = None  # anything else on PE invalidates
                    if pending_sync is not None:
                        si = inst.sync_info
                        if si is None:
                            import concourse.mybir as mybir

                            inst.sync_info = mybir.SyncInfo(
                                on_wait=list(pending_sync[0]),
                                on_update=list(pending_sync[1]),
                            )
                        else:
                            si.on_wait[:0] = pending_sync[0]
                            si.on_update.extend(pending_sync[1])
                        pending_sync = None
                out.append(inst)
            assert pending_sync is None, "dangling sync from deleted trailing LDW"
            bb.instructions.clear()
            for i in out:
                bb.add_instruction(i)
    return n_del


def _split_sync_waits(nc, mybir, max_waits=1):
    """This container's walrus only accepts 1 sync-wait per instruction;
    move excess waits onto preceding same-engine NoOps."""
    for f in nc.m.functions:
        for bb in f.blocks:
            out = []
            for inst in bb.instructions:
                si = inst.sync_info
                if si is not None and len(si.on_wait) > max_waits:
                    waits = list(si.on_wait)
                    extra, keep = waits[:-max_waits], waits[-max_waits:]
                    for i in range(0, len(extra), max_waits):
                        nop = mybir.InstNoOp(
                            name=nc.get_next_instruction_name(),
                            engine=inst.engine,
                            sync_info=mybir.SyncInfo(
                                on_wait=extra[i : i + max_waits], on_update=[]
                            ),
                        )
                        out.append(nop)
                    si.on_wait = keep
                out.append(inst)
            bb.instructions.clear()
            for i in out:
                bb.add_instruction(i)


def _build_nc():
    import concourse.bass as bass
    import concourse.mybir as mybir
    from concourse.tile import TileContext

    fp32 = mybir.dt.float32
    bf16 = mybir.dt.bfloat16
    fp8e4 = mybir.dt.float8e4
    DR = mybir.MatmulPerfMode.DoubleRow
    RELU = mybir.ActivationFunctionType.Relu
    ADD = mybir.AluOpType.add
    MAX = mybir.AluOpType.max

    nc = bass.Bass(trn_type="TRN2", num_devices=N_CORES)
    c4_d = nc.dram_tensor("c4", [NPAIR, 128, 4096], bf16, kind="ExternalInput")
    ctop_d = nc.dram_tensor("ctop", [128, 2048], bf16, kind="ExternalInput")
    wu_d = nc.dram_tensor("wu", [128, 512], bf16, kind="ExternalInput")
    whl_d = nc.dram_tensor("whl", [128, 128], bf16, kind="ExternalInput")
    whr_d = nc.dram_tensor("whr", [128, 128], bf16, kind="ExternalInput")
    whu_d = nc.dram_tensor("whu", [128, 128], bf16, kind="ExternalInput")
    whlr_d = nc.dram_tensor("whlr", [128, 256], fp8e4, kind="ExternalInput")
    bh2_d = nc.dram_tensor("bh2", [128, 1], fp32, kind="ExternalInput")
    out_d = nc.dram_tensor("out", [128, NPAIR], fp32, kind="ExternalOutput")

    # greedy act-engine balancing: est busy-ns per (scalar, vector)
    act_est = [0.0, 0.0]

    with TileContext(nc) as tc:
        with (
            tc.tile_pool(name="wpool", bufs=1) as wpool,
            tc.tile_pool(name="c4pool", bufs=3) as c4pool,
            tc.tile_pool(name="uspool", bufs=2) as uspool,
            tc.tile_pool(name="utpool", bufs=1) as utpool,
            tc.tile_pool(name="e12pool", bufs=3) as e12pool,
            tc.tile_pool(name="e11pool", bufs=3) as e11pool,
            tc.tile_pool(name="shpool", bufs=1) as shpool,
            tc.tile_pool(name="pspool", bufs=4, space="PSUM") as pspool,
        ):
            wu_sb = wpool.tile([128, 512], bf16, tag="wu")

            def wu_strip(t):
                return wu_sb[:, 128 * t : 128 * (t + 1)]
            whl_sb = wpool.tile([128, 128], bf16, tag="whl")
            whr_sb = wpool.tile([128, 128], bf16, tag="whr")
            whu_sb = wpool.tile([128, 128], bf16, tag="whu")
            whlr_sb = wpool.tile([128, 256], fp8e4, tag="whlr")
            bh_sb = wpool.tile([128, 1], fp32, tag="bh")
            ctop_sb = wpool.tile([128, 2048], bf16, tag="ctop")
            c4_sbs = [None] * NPAIR

            def dma_c4(p):
                # ~1MB per pair: split across two idle-engine queues (a
                # single queue moves ~100GB/s -> 10.4us for the full tile,
                # which is most of a 12.5us pair round).
                c4_sbs[p] = c4pool.tile([128, 4096], bf16, tag="c4", name=f"c4_{p}")
                nc.sync.dma_start(c4_sbs[p][:, 0:2048], c4_d.ap()[p][:, 0:2048])
                nc.gpsimd.dma_start(
                    c4_sbs[p][:, 2048:4096], c4_d.ap()[p][:, 2048:4096]
                )

            # Head: there is a ~7.3us fixed preamble before any DMA trigger
            # can fire and ~3-4us fixed latency per queue's first transfer,
            # so the first data can't land before ~11.2us. The play is to
            # pipeline: each queue delivers a stream of 512-col (128KB,
            # ~1.3us) chunks in exactly the order the PE consumes them.
            # Block b (cols 1024b:1024(b+1)) is read by groups 4b..4b+3;
            # sync carries the h0-half chunks, gpsimd the h1-halves, scalar
            # the weights (wu first, then whlr/whu/bh for pair-0 h-tiles).
            c4h = []
            for b in range(4):
                s = c4pool.tile([128, 512], bf16, tag=f"c4s{b}", name=f"c4s{b}", bufs=1)
                g = c4pool.tile([128, 512], bf16, tag=f"c4g{b}", name=f"c4g{b}", bufs=1)
                c4h.append((s, g))
            nc.scalar.dma_start(wu_sb[:], wu_d.ap())
            for b in range(4):
                nc.sync.dma_start(c4h[b][0][:], c4_d.ap()[0][:, 1024 * b : 1024 * b + 512])
                nc.gpsimd.dma_start(
                    c4h[b][1][:], c4_d.ap()[0][:, 1024 * b + 512 : 1024 * b + 1024]
                )
            nc.scalar.dma_start(whlr_sb[:], whlr_d.ap())
            nc.scalar.dma_start(whu_sb[:], whu_d.ap())
            nc.scalar.dma_start(bh_sb[:], bh2_d.ap())
            nc.gpsimd.dma_start(whl_sb[:], whl_d.ap())
            nc.gpsimd.dma_start(whr_sb[:], whr_d.ap())

            def act_relu(dst_ap, src_ap, cols, bias, engine=None):
                """relu(src [+ bias]) -> dst on the least-loaded act engine.
                engine=0/1 pins scalar/vector (tiny tail tiles stay on one
                engine to avoid cross-engine semaphore hops)."""
                cs = act_est[0] + 120.0 + 0.97 * cols
                cv = act_est[1] + 140.0 + 1.06 * cols
                if engine is None:
                    engine = 0 if cs <= cv else 1
                if engine == 0:
                    act_est[0] = cs
                    if bias is None:
                        nc.scalar.activation(dst_ap, src_ap, RELU)
                    else:
                        nc.scalar.activation(dst_ap, src_ap, RELU, bias=bias)
                else:
                    act_est[1] = cv
                    if bias is None:
                        nc.vector.tensor_scalar(dst_ap, src_ap, 0.0, None, MAX)
                    else:
                        nc.vector.tensor_scalar(dst_ap, src_ap, bias, 0.0, ADD, MAX)

            def u_units(src_of, dst_of, pname, order):
                """One thunk per fc_u group (1024 cols: 2 matmuls + act).
                Dense-label full-K form: tiled 18-row matmuls dual-issue
                (0.26 ns/col alternating strips) but drop the PE clock ~2x
                via the DVFS governor, which costs far more on the fc_h side
                than they save (measured 207us vs 147us).
                src_of: (g, h) -> (tile, col_base) for the g-th group's h-th
                512-col half. dst_of: g -> (tile, col_base)."""

                def mk(g):
                    def emit():
                        ps = pspool.tile(
                            [128, 1024], fp32, tag="ps", name=f"ups_{pname}_{g}"
                        )
                        for h in range(2):
                            src_sb, cc = src_of(g, h)
                            nc.tensor.matmul(
                                ps[:, 512 * h : 512 * (h + 1)],
                                wu_strip(g % 4),
                                src_sb[:, cc : cc + 512],
                                start=True,
                                stop=True,
                            )
                        dst_tile, dc = dst_of(g)
                        act_relu(dst_tile[:, dc : dc + 1024], ps[:, 0:1024], 1024, None)

                    return emit

                return [mk(g) for g in order]

            def h_tile(
                prev, prev_base, u_ap, u_base, dst, dst_base, w, bname, dr, eng=None
            ):
                """fc_h for up to TWO 1024-col psum tiles (w<=2048), fused so
                each stationary is loaded once: all DoubleRow LR matmuls
                first, then all bf16 U matmuls + acts (a weight switch costs
                a ~100ns PE issue bubble; dedup merges adjacent same-sig
                LDWEIGHTS).
                dr=True (levels 12..4): prev is stored fp8; one DoubleRow
                matmul contracts hL and hR together (halves = the even/odd
                emb columns, same issue cost as a single bf16 matmul), then
                the bf16 U matmul accumulates -> 2 PE cycles/col instead
                of 3. dr=False (top levels): bf16 L,L,R,R,U,U per tile."""
                tiles = []  # (ps, t0, n_cols)
                for t0 in range(0, w, 1024):
                    wt = min(1024, w - t0)
                    ps = pspool.tile(
                        [128, 1024], fp32, tag="ps", name=f"hps_{bname}_{t0}"
                    )
                    tiles.append((ps, t0, wt))
                if dr:
                    # U first: its moving data (u stream) is a round old,
                    # while the DR reads the JUST-written previous level.
                    # The U matmuls + LDWEIGHTS (~0.5us) then execute in the
                    # shadow of the previous level's act completion instead
                    # of the PE stalling on it before doing anything.
                    for ps, t0, wt in tiles:
                        for h0 in range(t0, t0 + wt, 512):
                            n = min(512, t0 + wt - h0)
                            nc.tensor.matmul(
                                ps[:, h0 - t0 : h0 - t0 + n],
                                whu_sb[:],
                                u_ap[:, u_base + h0 : u_base + h0 + n],
                                start=True,
                                stop=False,
                            )
                    for ps, t0, wt in tiles:
                        for h0 in range(t0, t0 + wt, 512):
                            n = min(512, t0 + wt - h0)
                            mv = prev[
                                :, prev_base + 2 * h0 : prev_base + 2 * h0 + 2 * n
                            ].rearrange("p (n two) -> p two n", two=2)
                            nc.tensor.matmul(
                                ps[:, h0 - t0 : h0 - t0 + n],
                                whlr_sb[:].rearrange("p (two m) -> p two m", two=2),
                                mv,
                                start=False,
                                stop=True,
                                perf_mode=DR,
                            )
                        act_relu(
                            dst[:, dst_base + t0 : dst_base + t0 + wt],
                            ps[:, 0:wt],
                            wt,
                            bh_sb[:],
                            eng,
                        )
                else:
                    for w_sb, kind in ((whu_sb, "U"), (whl_sb, "L"), (whr_sb, "R")):
                        for ps, t0, wt in tiles:
                            for h0 in range(t0, t0 + wt, 512):
                                n = min(512, t0 + wt - h0)
                                if kind == "L":
                                    mv = prev[
                                        :,
                                        prev_base
                                        + 2 * h0 : prev_base
                                        + 2 * h0
                                        + 2 * n : 2,
                                    ]
                                elif kind == "R":
                                    mv = prev[
                                        :,
                                        prev_base
                                        + 2 * h0
                                        + 1 : prev_base
                                        + 2 * h0
                                        + 2 * n : 2,
                                    ]
                                else:
                                    mv = u_ap[:, u_base + h0 : u_base + h0 + n]
                                nc.tensor.matmul(
                                    ps[:, h0 - t0 : h0 - t0 + n],
                                    w_sb[:],
                                    mv,
                                    start=(kind == "U"),
                                    stop=(kind == "R"),
                                )
                    for ps, t0, wt in tiles:
                        act_relu(
                            dst[:, dst_base + t0 : dst_base + t0 + wt],
                            ps[:, 0:wt],
                            wt,
                            bh_sb[:],
                            eng,
                        )

            def h_block(
                prev, prev_base, u_ap, u_base, dst, dst_base, ncols, bname, dr, eng=None
            ):
                """One fc_h stretch as a sequence of 1024-col tiles."""
                for c0 in range(0, ncols, 1024):
                    w = min(1024, ncols - c0)
                    h_tile(
                        prev,
                        prev_base + 2 * c0,
                        u_ap,
                        u_base + c0,
                        dst,
                        dst_base + c0,
                        w,
                        f"{bname}_{c0}",
                        dr,
                        eng,
                    )

            # emb at levels 13(us_leaf)..9 is stored fp8e4 (only ever read as
            # DoubleRow LR moving data); emb8 and above stay bf16.
            emb10sh = shpool.tile([128, 8192], fp8e4, tag="e10")
            usms = [None] * NPAIR
            usls = [None] * NPAIR
            utop = utpool.tile([128, UTOP_PAD], bf16, tag="utop")

            def h_units(p):
                """Per-tile thunks for pair p's levels 12..10 (7 tiles)."""
                usm, usl = usms[p], usls[p]
                emb12 = e12pool.tile([128, 4096], fp8e4, tag="e12", name=f"e12_{p}")
                emb11 = e11pool.tile([128, 2048], fp8e4, tag="e11", name=f"e11_{p}")
                units = []
                for c in range(4):
                    units.append(
                        lambda c=c: h_tile(
                            usl,
                            2048 * c,
                            usm,
                            UB12 + 1024 * c,
                            emb12,
                            1024 * c,
                            1024,
                            f"l12_{p}_{c}",
                            True,
                        )
                    )
                for c in range(2):
                    units.append(
                        lambda c=c: h_tile(
                            emb12,
                            2048 * c,
                            usm,
                            UB11 + 1024 * c,
                            emb11,
                            1024 * c,
                            1024,
                            f"l11_{p}_{c}",
                            True,
                        )
                    )
                units.append(
                    lambda: h_tile(
                        emb11, 0, usm, UB10, emb10sh, 1024 * p, 1024, f"l10_{p}", True
                    )
                )
                return units

            # ---- blended body: u-tiles of pair p interleaved ~2:1 with
            # h-tiles of pair p-1 (whose inputs are fully materialized), so
            # the act engines see a demand below their combined supply and
            # the PE never waits on psum recycling. ----
            # pair 0 consumes its head chunks in landing order (block-major
            # matches the two queue streams); other pairs put the lo-half
            # groups first (the sync-queue half of the prefetch lands a
            # round early, the gpsimd half carries the weights backlog).
            order_p0 = [0, 1, 2, 3, 4, 5, 6, 7, 8, 9, 10, 11, 12, 13, 14]

            def src_p0(g, h):
                return c4h[g // 4][h], 0

            def dst_pair(p):
                def dst_of(g):
                    if g <= 6:
                        return usms[p], 1024 * g
                    return usls[p], 1024 * (g - 7)

                return dst_of

            # adjacent same-strip pairs share one LDWEIGHTS via dedup; the
            # lo-half (c4 cols 0:2048) groups go first to match the split
            # prefetch arrival
            order_rest = [3, 7, 0, 4, 1, 5, 2, 6, 8, 12, 9, 13, 10, 14, 11]
            pend_h = []
            for p in range(NPAIR):
                # prefetch one round ahead: keeps the contended head
                # window (8 cores start their DMAs simultaneously) free
                # for the data the first rounds actually need
                if p + 1 < NPAIR:
                    dma_c4(p + 1)
                if p == 2:
                    nc.sync.dma_start(ctop_sb[:, 0:1024], ctop_d.ap()[:, 0:1024])
                if p == 3:
                    nc.gpsimd.dma_start(
                        ctop_sb[:, 1024:2048], ctop_d.ap()[:, 1024:2048]
                    )
                usms[p] = uspool.tile([128, USM], bf16, tag="usm", name=f"usm{p}")
                usls[p] = uspool.tile([128, USL], fp8e4, tag="usl", name=f"usl{p}")
                if p == 0:
                    uu = u_units(src_p0, dst_pair(0), "p0", order_p0)
                else:
                    src = c4_sbs[p]
                    uu = u_units(
                        lambda g, h, s=src: (s, 1024 * (g // 4) + 512 * h),
                        dst_pair(p),
                        f"p{p}",
                        order_rest,
                    )
                nu = 2  # 15 u : 7 h per pair ~ 2:1
                ui = hi = 0
                # h-tile first in each blend slot: its inputs are a full
                # round old, while the round-opening u-burst would otherwise
                # outrun the act engines right after the previous round's
                # trailing u-burst.
                while ui < len(uu) or hi < len(pend_h):
                    if hi < len(pend_h):
                        pend_h[hi]()
                        hi += 1
                    for _ in range(nu):
                        if ui < len(uu):
                            uu[ui]()
                            ui += 1
                pend_h = h_units(p) if p < NPAIR else []

            # ---- final phase: pair-7 h units, utop u groups, and the big
            # tail levels 9..8 hand-interleaved so the level chains run in
            # the act-latency shadow of the utop round, with the top-level
            # u groups (0..2) filling the PE during chain latencies.
            # utop group g covers cols 1024g..1024g+1024; level 9 needs
            # cols 4088:8184 (groups 3..7), level 8 needs 2040:4088
            # (groups 1..3), levels <=7 need 0:2040 (groups 0..2). ----
            uu = u_units(
                lambda g, h: (ctop_sb, 1024 * (g // 4) + 512 * h),
                lambda g: (utop, 1024 * g),
                "top",
                [3, 4, 5, 7, 6, 0, 1, 2],
            )
            h7 = pend_h  # pair 7: l12 x4, l11 x2, l10
            emb_sh = {}
            for i in range(9, 0, -1):
                emb_sh[i] = shpool.tile(
                    [128, 8 * 2**i],
                    fp8e4 if i >= 5 else bf16,
                    tag=f"esh{i}",
                    name=f"esh{i}",
                )

            def lvl(i, chain, nchain=2):
                half = 8 * 2**i // nchain
                prev = emb_sh[i + 1] if i < 9 else emb10sh
                h_block(
                    prev,
                    chain * 2 * half,
                    utop,
                    8 * (2**i - 1) + chain * half,
                    emb_sh[i],
                    chain * half,
                    half,
                    f"b{i}_{chain}",
                    i >= 4,
                    chain % 2 if i <= 6 else None,
                )

            seq = [
                h7[0], uu[0], h7[1], uu[1], h7[2], uu[2], h7[3], uu[3],
                h7[4], uu[4], h7[5], h7[6],
                lambda: lvl(9, 0), uu[5], uu[6],
                lambda: lvl(9, 1), uu[7],
                lambda: lvl(8, 0), lambda: lvl(8, 1),
            ]
            for s in seq:
                s()
            for i in range(7, 0, -1):
                nchain = 4 if i >= 4 else (2 if i >= 2 else 1)
                for chain in range(nchain):
                    lvl(i, chain, nchain)

            # ---- level 0: root ----
            roots = wpool.tile([128, NPAIR], fp32, tag="roots")
            ps = pspool.tile([128, 1024], fp32, tag="ps", name="root_ps")
            o = ps[:, 0:NPAIR]
            nc.tensor.matmul(o, whl_sb[:], emb_sh[1][:, 0:16:2], start=True, stop=False)
            nc.tensor.matmul(o, whr_sb[:], emb_sh[1][:, 1:16:2], start=False, stop=False)
            nc.tensor.matmul(o, whu_sb[:], utop[:, 0:NPAIR], start=False, stop=True)
            nc.scalar.activation(roots[:], o, RELU, bias=bh_sb[:])
            # out trigger on scalar: queued right behind the root act on the
            # same engine, skipping a cross-engine semaphore hop at the end
            nc.scalar.dma_start(out_d.ap(), roots[:])

    _dedup_ldweights(nc)
    _split_sync_waits(nc, mybir)
    return nc


_NC_CACHE = None
LAST_RESULTS = None


def kernel(contents, children, Wu, bu, Wh, bh):
    global _NC_CACHE, LAST_RESULTS
    contents = np.asarray(contents, dtype=np.float32)
    children = np.asarray(children)
    Wu = np.asarray(Wu, dtype=np.float32)
    bu = np.asarray(bu, dtype=np.float32)
    Wh = np.asarray(Wh, dtype=np.float32)
    bh = np.asarray(bh, dtype=np.float32)

    regular = (
        contents.shape == (B, N_NODES, F)
        and children.shape == (N_INNER, 2)
        and np.array_equal(
            np.asarray(children, dtype=np.int64).ravel(), np.arange(N_INNER * 2)
        )
    )
    if not regular:
        # Safety net for non-arange children: exact numpy fallback.
        return _np_reference(contents, children, Wu, bu, Wh, bh)

    from concourse.bass_utils import run_bass_kernel_spmd

    if _NC_CACHE is None:
        _NC_CACHE = _build_nc()
    nc = _NC_CACHE

    wts = _prep_weights(Wu, bu, Wh, bh)
    in_maps = []
    for k in range(N_CORES):
        m = _prep_core_inputs(contents[JPC * k : JPC * (k + 1)])
        m.update(wts)
        in_maps.append(m)

    res = run_bass_kernel_spmd(
        nc,
        in_maps,
        core_ids=list(range(N_CORES)),
        trace=bool(os.environ.get("BASS_TRACE")),
    )
    LAST_RESULTS = res

    out = np.empty((B, H), dtype=np.float32)
    for k in range(N_CORES):
        r = res.results[k]["out"].reshape(2, 64, NPAIR)  # [half, h, pair]
        out[JPC * k : JPC * (k + 1)] = np.transpose(r, (2, 0, 1)).reshape(JPC, H)
    return out



# revision 65
# speedup vs baseline: 1.0006x; 1.0006x over previous
"""Trainium2 Bass kernel for GRNNTransformSimple (bottom-up binary-tree GRNN).

Computation (per jet): heap-layout complete binary tree, DEPTH=14.
  u_k   = relu(contents_k @ Wu + bu)                         (all nodes)
  emb_k = u_k                                                (leaves)
  emb_k = relu(hL @ Wh[:64] + hR @ Wh[64:128] + u_k @ Wh[128:] + bh)  (inner)
Output: root emb, [B, 64].

Mapping (8 NeuronCores, data-parallel over B=128 jets, 16 jets/core):
 - 2 jets packed per 128 SBUF partitions (jet A on partitions 0-63, jet B on
   64-127) with block-diagonal weights -> all engines run 128 partitions wide.
 - fc_u biases folded into the matmul via a constant-one input row (K=18).
 - The "irregular" child gather is regular for arange children: children of
   level-i node j are nodes 2j, 2j+1 of level i+1, i.e. a stride-2 column
   slice of the level-(i+1) embedding buffer.

Performance notes (measured on TRN2; matmul issue cost is ~0.42 ns per
output column for bf16 K=128 AND for fp8 DoubleRow — which contracts two
K=128 halves in one pass; act relus are limited to ScalarE+VectorE at
~2.0 col/ns combined since GPSIMD and DMA cannot touch PSUM, and PSUM
reads lock the DVE to its 1x mode):
 - fc_h levels 12..4 fuse the hL/hR matmuls into ONE fp8e4m3 DoubleRow
   matmul: the moving AP's two "halves" are the even/odd emb columns of
   the level below (a stride-2 rearrange of the existing layout), the
   stationary is [blockdiag(WhL) | blockdiag(WhR)]. 2 PE cycles/col
   instead of 3; the u path and levels 3..0 stay bf16. Per tile the U
   matmul issues FIRST (start=True) and the DR accumulates (stop=True):
   U's moving data is a round old while DR reads the just-written level
   below, so the U work hides in the shadow of that act completing. emb at levels
   13..5 is stored fp8 (only ever read as DR moving data); quantization
   noise injected below level ~4 washes out through the tree (measured
   rel_rms 4.8e-3 vs 3.8e-3 all-bf16, tolerance 2e-2).
 - fc_u is output-bound (each 128-partition psum column = 2 jets x 64H,
   full density) -> stays bf16; DoubleRow at K_phys=18 measured 2x SLOWER
   per output column, so it is NOT used for fc_u. Stationaries stay
   zero-padded to full K=128 ("dense-label") to keep the DVFS governor
   from downclocking on low-occupancy matmuls.
 - Head: ~7.3us fixed preamble before any DMA trigger can fire plus
   ~3.2us fixed first-transfer latency per queue -> first data lands
   ~11us in regardless of size. Each of the three DMA-capable queues
   (sync/scalar/gpsimd) delivers a stream of 512-col chunks in exactly
   the PE's consumption order (pair-0 h0-halves on sync, h1-halves on
   gpsimd, weights on scalar), so the PE never re-stalls after the first
   landing. Steady-state c4[p] prefetch is split sync/gpsimd (a single
   queue moves ~100GB/s = most of a round for the 1MB pair tile).
 - Blended emission: h-tile of pair p-1 first in each slot, then 2
   u-tiles of pair p; adjacent same-strip u-pairs share one LDWEIGHTS
   via dedup (a weight switch costs a ~100ns PE issue bubble; fusing
   MORE tiles per stationary measured slower - psum pool pressure).
 - One shared 4-deep PSUM pool of 1024-col tiles (2048-col tiles with
   bufs=2 measured catastrophically slower: the 2-deep pipeline
   lockstepped the PE to act latency).
 - Greedy ns-balanced act assignment (scalar 120+0.97/col, vector
   140+1.06/col); tiny tail levels (<=6) pin each column-chain's acts
   to one engine (chain%2) so lineages run in parallel without
   cross-engine semaphore hops inside a lineage.
 - Tail: pair-7 h units, utop u groups (level-9 inputs first) and the
   level 9..8 chains hand-interleaved so the big tail levels run in the
   act-latency shadow of the utop round; levels 7..1 as two independent
   column-chains (chain c of level i feeds chain c of level i-1), root
   act + out DMA on scalar.
"""

import os
import sys

sys.path.insert(0, "/opt/trn_rl_repo")

import ml_dtypes
import numpy as np

DEPTH = 14
B = 128
F = 8
H = 64
N_NODES = 2**DEPTH - 1  # 16383
N_INNER = 2 ** (DEPTH - 1) - 1  # 8191
N_CORES = 8
JPC = 16  # jets per core
NPAIR = 8  # jet pairs per core

BF16 = ml_dtypes.bfloat16
E4M3 = ml_dtypes.float8_e4m3

# u_stream layout per pair (columns): levels 10,11,12 inner nodes in heap
# order (bf16 "usm" tile, 7168 cols), then all leaves in heap order (fp8
# "usl" tile, 8192 cols; leaves only feed the level-12 DoubleRow LR matmul).
UB10, UB11, UB12 = 0, 1024, 3072  # level bases inside usm
USM = 7168
USL = 8192
USTREAM = 15360  # 1024 + 2048 + 4096 + 8192
NGRP = 15  # 15 groups x 1024 cols
# u_top: levels 0..9, column order [level][pair][node]
UTOP_COLS = 8184  # 8 * 1023
UTOP_PAD = 8192


def _np_reference(contents, children, Wu, bu, Wh, bh):
    emb = None
    for i in range(DEPTH - 1, -1, -1):
        off, n = 2**i - 1, 2**i
        u = np.maximum(contents[:, off : off + n] @ Wu + bu, 0)
        if emb is None:
            emb = u
        else:
            ch = children[off : off + n] - 2 * off
            hL = emb[:, ch[:, 0]]
            hR = emb[:, ch[:, 1]]
            emb = np.maximum(
                hL @ Wh[:H] + hR @ Wh[H : 2 * H] + u @ Wh[2 * H :] + bh, 0
            )
    return emb.reshape(emb.shape[0], -1).astype(np.float32)


def _prep_core_inputs(contents):
    """contents: [16, 16383, 8] f32 for one core.
    Returns dict of per-core device input arrays."""
    c4 = np.zeros((NPAIR, 128, 4096), dtype=BF16)
    big_T = np.ascontiguousarray(
        np.transpose(contents[:, 1023:16383, :], (0, 2, 1))
    )  # [16, 8, 15360]
    for p in range(NPAIR):
        S = np.empty((18, USTREAM), dtype=np.float32)
        S[0:8] = big_T[2 * p]
        S[8] = 1.0
        S[9:17] = big_T[2 * p + 1]
        S[17] = 1.0
        Sb = S.astype(BF16)
        for g in range(NGRP):
            t = g % 4
            cc = 1024 * (g // 4)
            c4[p, 32 * t : 32 * t + 18, cc : cc + 1024] = Sb[
                :, 1024 * g : 1024 * (g + 1)
            ]

    # u_top stream: levels 0..9, [level][pair][node]
    tops = np.empty((18, UTOP_COLS), dtype=np.float32)
    colptr = 0
    cT = np.transpose(contents, (0, 2, 1))  # [16, 8, 16383]
    for i in range(10):
        off, n = 2**i - 1, 2**i
        for p in range(NPAIR):
            tops[0:8, colptr : colptr + n] = cT[2 * p][:, off : off + n]
            tops[8, colptr : colptr + n] = 1.0
            tops[9:17, colptr : colptr + n] = cT[2 * p + 1][:, off : off + n]
            tops[17, colptr : colptr + n] = 1.0
            colptr += n
    assert colptr == UTOP_COLS
    ctop = np.zeros((128, 2048), dtype=BF16)
    tb = np.zeros((18, UTOP_PAD), dtype=BF16)
    tb[:, :UTOP_COLS] = tops.astype(BF16)
    for g in range(8):
        t = g % 4
        cc = 1024 * (g // 4)
        ctop[32 * t : 32 * t + 18, cc : cc + 1024] = tb[:, 1024 * g : 1024 * (g + 1)]
    return {"c4": c4, "ctop": ctop}


def _prep_weights(Wu, bu, Wh, bh):
    wu2 = np.zeros((18, 128), dtype=np.float32)
    wu2[0:8, 0:64] = Wu
    wu2[8, 0:64] = bu
    wu2[9:17, 64:128] = Wu
    wu2[17, 64:128] = bu
    # Four full-K stationaries (one per 32-row strip): rows outside the
    # strip are zero so the other strips' data in the moving columns
    # contributes nothing. Full-K keeps the mm "dense" (128x128) from the
    # DVFS governor's perspective.
    wu_dram = np.zeros((4, 128, 128), dtype=BF16)
    for t in range(4):
        wu_dram[t, 32 * t : 32 * t + 18, :] = wu2.astype(BF16)
    wu_dram = wu_dram.transpose(1, 0, 2).reshape(128, 512)

    def blockdiag(Wx, dt=BF16):
        out = np.zeros((128, 128), dtype=np.float32)
        out[0:64, 0:64] = Wx
        out[64:128, 64:128] = Wx
        return out.astype(dt)

    whl = blockdiag(Wh[0:H])
    whr = blockdiag(Wh[H : 2 * H])
    whu = blockdiag(Wh[2 * H : 3 * H])
    # fp8 DoubleRow stationary for levels 12..4: half0 = blockdiag(WhL),
    # half1 = blockdiag(WhR); the two halves contract the even/odd emb
    # columns of the level below in a single PE pass.
    whlr = np.concatenate(
        [blockdiag(Wh[0:H], E4M3), blockdiag(Wh[H : 2 * H], E4M3)], axis=1
    )
    bh2 = np.concatenate([bh, bh]).astype(np.float32).reshape(128, 1)
    return {
        "wu": wu_dram,
        "whl": whl,
        "whr": whr,
        "whu": whu,
        "whlr": whlr,
        "bh2": bh2,
    }


def _dedup_ldweights(nc):
    """Delete an LDWEIGHTS whose signature matches the previous PE weight
    load when only instructions that cannot disturb the stationary operand
    (MATMULs, NoOps, semaphore ops) execute in between: the PE keeps the
    stationary resident, so load-once-matmul-many is safe. Sync info of
    deleted loads is merged into the following PE instruction."""
    n_del = 0
    transparent = ("InstMatmult", "InstNoOp", "InstEventSemaphore")
    for f in nc.m.functions:
        for bb in f.blocks:
            last_sig = None
            pending_sync = None
            out = []
            for inst in bb.instructions:
                tn = type(inst).__name__
                if str(getattr(inst, "engine", "")) == "EngineType.PE":
                    if tn == "InstLdweights":
                        a = inst.ins[0]
                        sig = (
                            getattr(a, "memref", None),
                            getattr(a, "offset", None),
                            str(getattr(a, "ap", None)),
                            str(inst.tile_position),
                            str(inst.tile_size),
                            str(inst.perf_mode),
                            str(inst.is_transpose),
                        )
                        if sig == last_sig:
                            n_del += 1
                            si = inst.sync_info
                            if si is not None and (si.on_wait or si.on_update):
                                if pending_sync is None:
                                    pending_sync = ([], [])
                                pending_sync[0].extend(si.on_wait)
                                pending_sync[1].extend(si.on_update)
                            continue  # drop this instruction
                        last_sig = sig
                    elif tn not in transparent:
                        last_sig = None  # anything else on PE invalidates
                    if pending_sync is not None:
                        si = inst.sync_info
                        if si is None:
                            import concourse.mybir as mybir

                            inst.sync_info = mybir.SyncInfo(
                                on_wait=list(pending_sync[0]),
                                on_update=list(pending_sync[1]),
                            )
                        else:
                            si.on_wait[:0] = pending_sync[0]
                            si.on_update.extend(pending_sync[1])
                        pending_sync = None
                out.append(inst)
            assert pending_sync is None, "dangling sync from deleted trailing LDW"
            bb.instructions.clear()
            for i in out:
                bb.add_instruction(i)
    return n_del


def _split_sync_waits(nc, mybir, max_waits=1):
    """This container's walrus only accepts 1 sync-wait per instruction;
    move excess waits onto preceding same-engine NoOps."""
    for f in nc.m.functions:
        for bb in f.blocks:
            out = []
            for inst in bb.instructions:
                si = inst.sync_info
                if si is not None and len(si.on_wait) > max_waits:
                    waits = list(si.on_wait)
                    extra, keep = waits[:-max_waits], waits[-max_waits:]
                    for i in range(0, len(extra), max_waits):
                        nop = mybir.InstNoOp(
                            name=nc.get_next_instruction_name(),
                            engine=inst.engine,
                            sync_info=mybir.SyncInfo(
                                on_wait=extra[i : i + max_waits], on_update=[]
                            ),
                        )
                        out.append(nop)
                    si.on_wait = keep
                out.append(inst)
            bb.instructions.clear()
            for i in out:
                bb.add_instruction(i)


def _build_nc():
    import concourse.bass as bass
    import concourse.mybir as mybir
    from concourse.tile import TileContext

    fp32 = mybir.dt.float32
    bf16 = mybir.dt.bfloat16
    fp8e4 = mybir.dt.float8e4
    DR = mybir.MatmulPerfMode.DoubleRow
    RELU = mybir.ActivationFunctionType.Relu
    ADD = mybir.AluOpType.add
    MAX = mybir.AluOpType.max

    nc = bass.Bass(trn_type="TRN2", num_devices=N_CORES)
    c4_d = nc.dram_tensor("c4", [NPAIR, 128, 4096], bf16, kind="ExternalInput")
    ctop_d = nc.dram_tensor("ctop", [128, 2048], bf16, kind="ExternalInput")
    wu_d = nc.dram_tensor("wu", [128, 512], bf16, kind="ExternalInput")
    whl_d = nc.dram_tensor("whl", [128, 128], bf16, kind="ExternalInput")
    whr_d = nc.dram_tensor("whr", [128, 128], bf16, kind="ExternalInput")
    whu_d = nc.dram_tensor("whu", [128, 128], bf16, kind="ExternalInput")
    whlr_d = nc.dram_tensor("whlr", [128, 256], fp8e4, kind="ExternalInput")
    bh2_d = nc.dram_tensor("bh2", [128, 1], fp32, kind="ExternalInput")
    out_d = nc.dram_tensor("out", [128, NPAIR], fp32, kind="ExternalOutput")

    # greedy act-engine balancing: est busy-ns per (scalar, vector)
    act_est = [0.0, 0.0]

    with TileContext(nc) as tc:
        with (
            tc.tile_pool(name="wpool", bufs=1) as wpool,
            tc.tile_pool(name="c4pool", bufs=3) as c4pool,
            tc.tile_pool(name="uspool", bufs=2) as uspool,
            tc.tile_pool(name="utpool", bufs=1) as utpool,
            tc.tile_pool(name="e12pool", bufs=3) as e12pool,
            tc.tile_pool(name="e11pool", bufs=3) as e11pool,
            tc.tile_pool(name="shpool", bufs=1) as shpool,
            tc.tile_pool(name="pspool", bufs=4, space="PSUM") as pspool,
        ):
            wu_sb = wpool.tile([128, 512], bf16, tag="wu")

            def wu_strip(t):
                return wu_sb[:, 128 * t : 128 * (t + 1)]
            whl_sb = wpool.tile([128, 128], bf16, tag="whl")
            whr_sb = wpool.tile([128, 128], bf16, tag="whr")
            whu_sb = wpool.tile([128, 128], bf16, tag="whu")
            whlr_sb = wpool.tile([128, 256], fp8e4, tag="whlr")
            bh_sb = wpool.tile([128, 1], fp32, tag="bh")
            ctop_sb = wpool.tile([128, 2048], bf16, tag="ctop")
            c4_sbs = [None] * NPAIR

            def dma_c4(p):
                # ~1MB per pair: split across two idle-engine queues (a
                # single queue moves ~100GB/s -> 10.4us for the full tile,
                # which is most of a 12.5us pair round).
                c4_sbs[p] = c4pool.tile([128, 4096], bf16, tag="c4", name=f"c4_{p}")
                nc.sync.dma_start(c4_sbs[p][:, 0:2048], c4_d.ap()[p][:, 0:2048])
                nc.gpsimd.dma_start(
                    c4_sbs[p][:, 2048:4096], c4_d.ap()[p][:, 2048:4096]
                )

            # Head: there is a ~7.3us fixed preamble before any DMA trigger
            # can fire and ~3-4us fixed latency per queue's first transfer,
            # so the first data can't land before ~11.2us. The play is to
            # pipeline: each queue delivers a stream of 512-col (128KB,
            # ~1.3us) chunks in exactly the order the PE consumes them.
            # Block b (cols 1024b:1024(b+1)) is read by groups 4b..4b+3;
            # sync carries the h0-half chunks, gpsimd the h1-halves, scalar
            # the weights (wu first, then whlr/whu/bh for pair-0 h-tiles).
            c4h = []
            for b in range(4):
                s = c4pool.tile([128, 512], bf16, tag=f"c4s{b}", name=f"c4s{b}", bufs=1)
                g = c4pool.tile([128, 512], bf16, tag=f"c4g{b}", name=f"c4g{b}", bufs=1)
                c4h.append((s, g))
            nc.scalar.dma_start(wu_sb[:], wu_d.ap())
            for b in range(4):
                nc.sync.dma_start(c4h[b][0][:], c4_d.ap()[0][:, 1024 * b : 1024 * b + 512])
                nc.gpsimd.dma_start(
                    c4h[b][1][:], c4_d.ap()[0][:, 1024 * b + 512 : 1024 * b + 1024]
                )
            nc.scalar.dma_start(whlr_sb[:], whlr_d.ap())
            nc.scalar.dma_start(whu_sb[:], whu_d.ap())
            nc.scalar.dma_start(bh_sb[:], bh2_d.ap())
            nc.gpsimd.dma_start(whl_sb[:], whl_d.ap())
            nc.gpsimd.dma_start(whr_sb[:], whr_d.ap())

            def act_relu(dst_ap, src_ap, cols, bias, engine=None):
                """relu(src [+ bias]) -> dst on the least-loaded act engine.
                engine=0/1 pins scalar/vector (tiny tail tiles stay on one
                engine to avoid cross-engine semaphore hops)."""
                cs = act_est[0] + 120.0 + 0.97 * cols
                cv = act_est[1] + 140.0 + 1.06 * cols
                if engine is None:
                    engine = 0 if cs <= cv else 1
                if engine == 0:
                    act_est[0] = cs
                    if bias is None:
                        nc.scalar.activation(dst_ap, src_ap, RELU)
                    else:
                        nc.scalar.activation(dst_ap, src_ap, RELU, bias=bias)
                else:
                    act_est[1] = cv
                    if bias is None:
                        nc.vector.tensor_scalar(dst_ap, src_ap, 0.0, None, MAX)
                    else:
                        nc.vector.tensor_scalar(dst_ap, src_ap, bias, 0.0, ADD, MAX)

            def u_units(src_of, dst_of, pname, order):
                """One thunk per fc_u group (1024 cols: 2 matmuls + act).
                Dense-label full-K form: tiled 18-row matmuls dual-issue
                (0.26 ns/col alternating strips) but drop the PE clock ~2x
                via the DVFS governor, which costs far more on the fc_h side
                than they save (measured 207us vs 147us).
                src_of: (g, h) -> (tile, col_base) for the g-th group's h-th
                512-col half. dst_of: g -> (tile, col_base)."""

                def mk(g):
                    def emit():
                        ps = pspool.tile(
                            [128, 1024], fp32, tag="ps", name=f"ups_{pname}_{g}"
                        )
                        for h in range(2):
                            src_sb, cc = src_of(g, h)
                            nc.tensor.matmul(
                                ps[:, 512 * h : 512 * (h + 1)],
                                wu_strip(g % 4),
                                src_sb[:, cc : cc + 512],
                                start=True,
                                stop=True,
                            )
                        dst_tile, dc = dst_of(g)
                        act_relu(dst_tile[:, dc : dc + 1024], ps[:, 0:1024], 1024, None)

                    return emit

                return [mk(g) for g in order]

            def h_tile(
                prev, prev_base, u_ap, u_base, dst, dst_base, w, bname, dr, eng=None
            ):
                """fc_h for up to TWO 1024-col psum tiles (w<=2048), fused so
                each stationary is loaded once: all DoubleRow LR matmuls
                first, then all bf16 U matmuls + acts (a weight switch costs
                a ~100ns PE issue bubble; dedup merges adjacent same-sig
                LDWEIGHTS).
                dr=True (levels 12..4): prev is stored fp8; one DoubleRow
                matmul contracts hL and hR together (halves = the even/odd
                emb columns, same issue cost as a single bf16 matmul), then
                the bf16 U matmul accumulates -> 2 PE cycles/col instead
                of 3. dr=False (top levels): bf16 L,L,R,R,U,U per tile."""
                tiles = []  # (ps, t0, n_cols)
                for t0 in range(0, w, 1024):
                    wt = min(1024, w - t0)
                    ps = pspool.tile(
                        [128, 1024], fp32, tag="ps", name=f"hps_{bname}_{t0}"
                    )
                    tiles.append((ps, t0, wt))
                if dr:
                    # U first: its moving data (u stream) is a round old,
                    # while the DR reads the JUST-written previous level.
                    # The U matmuls + LDWEIGHTS (~0.5us) then execute in the
                    # shadow of the previous level's act completion instead
                    # of the PE stalling on it before doing anything.
                    for ps, t0, wt in tiles:
                        for h0 in range(t0, t0 + wt, 512):
                            n = min(512, t0 + wt - h0)
                            nc.tensor.matmul(
                                ps[:, h0 - t0 : h0 - t0 + n],
                                whu_sb[:],
                                u_ap[:, u_base + h0 : u_base + h0 + n],
                                start=True,
                                stop=False,
                            )
                    for ps, t0, wt in tiles:
                        for h0 in range(t0, t0 + wt, 512):
                            n = min(512, t0 + wt - h0)
                            mv = prev[
                                :, prev_base + 2 * h0 : prev_base + 2 * h0 + 2 * n
                            ].rearrange("p (n two) -> p two n", two=2)
                            nc.tensor.matmul(
                                ps[:, h0 - t0 : h0 - t0 + n],
                                whlr_sb[:].rearrange("p (two m) -> p two m", two=2),
                                mv,
                                start=False,
                                stop=True,
                                perf_mode=DR,
                            )
                        act_relu(
                            dst[:, dst_base + t0 : dst_base + t0 + wt],
                            ps[:, 0:wt],
                            wt,
                            bh_sb[:],
                            eng,
                        )
                else:
                    for w_sb, kind in ((whu_sb, "U"), (whl_sb, "L"), (whr_sb, "R")):
                        for ps, t0, wt in tiles:
                            for h0 in range(t0, t0 + wt, 512):
                                n = min(512, t0 + wt - h0)
                                if kind == "L":
                                    mv = prev[
                                        :,
                                        prev_base
                                        + 2 * h0 : prev_base
                                        + 2 * h0
                                        + 2 * n : 2,
                                    ]
                                elif kind == "R":
                                    mv = prev[
                                        :,
                                        prev_base
                                        + 2 * h0
                                        + 1 : prev_base
                                        + 2 * h0
                                        + 2 * n : 2,
                                    ]
                                else:
                                    mv = u_ap[:, u_base + h0 : u_base + h0 + n]
                                nc.tensor.matmul(
                                    ps[:, h0 - t0 : h0 - t0 + n],
                                    w_sb[:],
                                    mv,
                                    start=(kind == "U"),
                                    stop=(kind == "R"),
                                )
                    for ps, t0, wt in tiles:
                        act_relu(
                            dst[:, dst_base + t0 : dst_base + t0 + wt],
                            ps[:, 0:wt],
                            wt,
                            bh_sb[:],
                            eng,
                        )

            def h_block(
                prev, prev_base, u_ap, u_base, dst, dst_base, ncols, bname, dr, eng=None
            ):
                """One fc_h stretch as a sequence of 1024-col tiles."""
                for c0 in range(0, ncols, 1024):
                    w = min(1024, ncols - c0)
                    h_tile(
                        prev,
                        prev_base + 2 * c0,
                        u_ap,
                        u_base + c0,
                        dst,
                        dst_base + c0,
                        w,
                        f"{bname}_{c0}",
                        dr,
                        eng,
                    )

            # emb at levels 13(us_leaf)..9 is stored fp8e4 (only ever read as
            # DoubleRow LR moving data); emb8 and above stay bf16.
            emb10sh = shpool.tile([128, 8192], fp8e4, tag="e10")
            usms = [None] * NPAIR
            usls = [None] * NPAIR
            utop = utpool.tile([128, UTOP_PAD], bf16, tag="utop")

            def h_units(p):
                """Per-tile thunks for pair p's levels 12..10 (7 tiles)."""
                usm, usl = usms[p], usls[p]
                emb12 = e12pool.tile([128, 4096], fp8e4, tag="e12", name=f"e12_{p}")
                emb11 = e11pool.tile([128, 2048], fp8e4, tag="e11", name=f"e11_{p}")
                units = []
                for c in range(4):
                    units.append(
                        lambda c=c: h_tile(
                            usl,
                            2048 * c,
                            usm,
                            UB12 + 1024 * c,
                            emb12,
                            1024 * c,
                            1024,
                            f"l12_{p}_{c}",
                            True,
                        )
                    )
                for c in range(2):
                    units.append(
                        lambda c=c: h_tile(
                            emb12,
                            2048 * c,
                            usm,
                            UB11 + 1024 * c,
                            emb11,
                            1024 * c,
                            1024,
                            f"l11_{p}_{c}",
                            True,
                        )
                    )
                units.append(
                    lambda: h_tile(
                        emb11, 0, usm, UB10, emb10sh, 1024 * p, 1024, f"l10_{p}", True
                    )
                )
                return units

            # ---- blended body: u-tiles of pair p interleaved ~2:1 with
            # h-tiles of pair p-1 (whose inputs are fully materialized), so
            # the act engines see a demand below their combined supply and
            # the PE never waits on psum recycling. ----
            # pair 0 consumes its head chunks in landing order (block-major
            # matches the two queue streams); other pairs put the lo-half
            # groups first (the sync-queue half of the prefetch lands a
            # round early, the gpsimd half carries the weights backlog).
            order_p0 = [0, 1, 2, 3, 4, 5, 6, 7, 8, 9, 10, 11, 12, 13, 14]

            def src_p0(g, h):
                return c4h[g // 4][h], 0

            def dst_pair(p):
                def dst_of(g):
                    if g <= 6:
                        return usms[p], 1024 * g
                    return usls[p], 1024 * (g - 7)

                return dst_of

            # adjacent same-strip pairs share one LDWEIGHTS via dedup; the
            # lo-half (c4 cols 0:2048) groups go first to match the split
            # prefetch arrival
            order_rest = [3, 7, 0, 4, 1, 5, 2, 6, 8, 12, 9, 13, 10, 14, 11]
            pend_h = []
            for p in range(NPAIR):
                # prefetch one round ahead: keeps the contended head
                # window (8 cores start their DMAs simultaneously) free
                # for the data the first rounds actually need
                if p + 1 < NPAIR:
                    dma_c4(p + 1)
                if p == 2:
                    nc.sync.dma_start(ctop_sb[:, 0:1024], ctop_d.ap()[:, 0:1024])
                if p == 3:
                    nc.gpsimd.dma_start(
                        ctop_sb[:, 1024:2048], ctop_d.ap()[:, 1024:2048]
                    )
                usms[p] = uspool.tile([128, USM], bf16, tag="usm", name=f"usm{p}")
                usls[p] = uspool.tile([128, USL], fp8e4, tag="usl", name=f"usl{p}")
                if p == 0:
                    uu = u_units(src_p0, dst_pair(0), "p0", order_p0)
                else:
                    src = c4_sbs[p]
                    uu = u_units(
                        lambda g, h, s=src: (s, 1024 * (g // 4) + 512 * h),
                        dst_pair(p),
                        f"p{p}",
                        order_rest,
                    )
                nu = 2  # 15 u : 7 h per pair ~ 2:1
                ui = hi = 0
                # h-tile first in each blend slot: its inputs are a full
                # round old, while the round-opening u-burst would otherwise
                # outrun the act engines right after the previous round's
                # trailing u-burst.
                while ui < len(uu) or hi < len(pend_h):
                    if hi < len(pend_h):
                        pend_h[hi]()
                        hi += 1
                    for _ in range(nu):
                        if ui < len(uu):
                            uu[ui]()
                            ui += 1
                pend_h = h_units(p) if p < NPAIR else []

            # ---- final phase: pair-7 h units, utop u groups, and the big
            # tail levels 9..8 hand-interleaved so the level chains run in
            # the act-latency shadow of the utop round, with the top-level
            # u groups (0..2) filling the PE during chain latencies.
            # utop group g covers cols 1024g..1024g+1024; level 9 needs
            # cols 4088:8184 (groups 3..7), level 8 needs 2040:4088
            # (groups 1..3), levels <=7 need 0:2040 (groups 0..2). ----
            uu = u_units(
                lambda g, h: (ctop_sb, 1024 * (g // 4) + 512 * h),
                lambda g: (utop, 1024 * g),
                "top",
                [3, 4, 5, 7, 6, 0, 1, 2],
            )
            h7 = pend_h  # pair 7: l12 x4, l11 x2, l10
            emb_sh = {}
            for i in range(9, 0, -1):
                emb_sh[i] = shpool.tile(
                    [128, 8 * 2**i],
                    fp8e4 if i >= 5 else bf16,
                    tag=f"esh{i}",
                    name=f"esh{i}",
                )

            def lvl(i, chain, nchain=2):
                half = 8 * 2**i // nchain
                prev = emb_sh[i + 1] if i < 9 else emb10sh
                h_block(
                    prev,
                    chain * 2 * half,
                    utop,
                    8 * (2**i - 1) + chain * half,
                    emb_sh[i],
                    chain * half,
                    half,
                    f"b{i}_{chain}",
                    i >= 4,
                    chain % 2 if i <= 6 else None,
                )

            seq = [
                h7[0], uu[0], h7[1], uu[1], h7[2], uu[2], h7[3], uu[3],
                h7[4], uu[4], h7[5], h7[6],
                lambda: lvl(9, 0), uu[5], uu[6],
                lambda: lvl(9, 1), uu[7],
                lambda: lvl(8, 0), lambda: lvl(8, 1),
            ]
            for s in seq:
                s()
            for i in range(7, 0, -1):
                nchain = 4 if i >= 4 else (2 if i >= 2 else 1)
                for chain in range(nchain):
                    lvl(i, chain, nchain)

            # ---- level 0: root ----
            roots = wpool.tile([128, NPAIR], fp32, tag="roots")
            ps = pspool.tile([128, 1024], fp32, tag="ps", name="root_ps")
            o = ps[:, 0:NPAIR]
            # U first: utop is long ready, emb_sh[1] is the hot dependency
            nc.tensor.matmul(o, whu_sb[:], utop[:, 0:NPAIR], start=True, stop=False)
            nc.tensor.matmul(o, whl_sb[:], emb_sh[1][:, 0:16:2], start=False, stop=False)
            nc.tensor.matmul(o, whr_sb[:], emb_sh[1][:, 1:16:2], start=False, stop=True)
            nc.scalar.activation(roots[:], o, RELU, bias=bh_sb[:])
            # out trigger on scalar: queued right behind the root act on the
            # same engine, skipping a cross-engine semaphore hop at the end
            nc.scalar.dma_start(out_d.ap(), roots[:])

    _dedup_ldweights(nc)
    _split_sync_waits(nc, mybir)
    return nc


_NC_CACHE = None
LAST_RESULTS = None


def kernel(contents, children, Wu, bu, Wh, bh):
    global _NC_CACHE, LAST_RESULTS
    contents = np.asarray(contents, dtype=np.float32)
    children = np.asarray(children)
    Wu = np.asarray(Wu, dtype=np.float32)
    bu = np.asarray(bu, dtype=np.float32)
    Wh = np.asarray(Wh, dtype=np.float32)
    bh = np.asarray(bh, dtype=np.float32)

    regular = (
        contents.shape == (B, N_NODES, F)
        and children.shape == (N_INNER, 2)
        and np.array_equal(
            np.asarray(children, dtype=np.int64).ravel(), np.arange(N_INNER * 2)
        )
    )
    if not regular:
        # Safety net for non-arange children: exact numpy fallback.
        return _np_reference(contents, children, Wu, bu, Wh, bh)

    from concourse.bass_utils import run_bass_kernel_spmd

    if _NC_CACHE is None:
        _NC_CACHE = _build_nc()
    nc = _NC_CACHE

    wts = _prep_weights(Wu, bu, Wh, bh)
    in_maps = []
    for k in range(N_CORES):
        m = _prep_core_inputs(contents[JPC * k : JPC * (k + 1)])
        m.update(wts)
        in_maps.append(m)

    res = run_bass_kernel_spmd(
        nc,
        in_maps,
        core_ids=list(range(N_CORES)),
        trace=bool(os.environ.get("BASS_TRACE")),
    )
    LAST_RESULTS = res

    out = np.empty((B, H), dtype=np.float32)
    for k in range(N_CORES):
        r = res.results[k]["out"].reshape(2, 64, NPAIR)  # [half, h, pair]
        out[JPC * k : JPC * (k + 1)] = np.transpose(r, (2, 0, 1)).reshape(JPC, H)
    return out

